# revision 13
# baseline (speedup 1.0000x reference)
"""Trainium2 Bass kernel for nn_CIntegration_3487513444382 (embedding_lookup).

Computation (per token): ct = concat(onehot(rgap,32), onehot(sgap,32),
onehot(pcount,32)); out = concat(vt * (ct @ W.T), ct).

Strategy: pure data parallel over the batch dim (64 -> 8 per core), with
all device-side tensors transposed to [feature, token] so the rel-err
budget (2e-2) can buy bandwidth: vt is fed as bf16, theta is stored as
bf16, and the one-hot tail is stored as fp8 (0/1 exact). Per core this
moves ~9.3MB instead of ~20MB of f32 traffic.

Per 512-token group: a tiny E3 matmul broadcasts the (offset) indices
to 96 partitions, a DVE compare against an iota column builds the
transposed one-hot in bf16 (an fp8 PE stream keeps the HAM cold), the
PE streams that one-hot through stationary W.T halves (bf16) to
produce Cct.T in PSUM, ScalarE copies it to SBUF as bf16, and a
2x-mode DVE multiply applies the vt gate. GpSimd down-copies the
one-hot to fp8 for the store. All PSUM tiles are single-bank (3 bc
bufs + 5 mm bufs = 8 banks) and the issue order is software-pipelined
one pair ahead so every engine stays fed and the PE holds its warm
2.4GHz p-state. All vt loads are issued up front.
"""
import numpy as np

import concourse.bass as bass
import concourse.tile as tile
from concourse import bacc, mybir
from concourse.bass_utils import run_bass_kernel_spmd

F32 = mybir.dt.float32
BF16 = mybir.dt.bfloat16
FP8 = mybir.dt.float8e4

N_CORES = 8
B, S, E = 64, 1024, 256
BPC = B // N_CORES          # 8 batches per core
NTOK = BPC * S              # 8192 tokens per core
NCH = NTOK // 128           # 64 chunks of 128 tokens
GTOK = 512                  # tokens per compute group
PTOK = 2 * GTOK             # tokens per pair (DMA batch)
NPAIR = NTOK // PTOK        # 8
NTOT = 96                   # one-hot width
EH = E // 128               # 2 e-halves

_NC = None


def _build_nc():
    nc = bacc.Bacc("TRN2", target_bir_lowering=False, debug=False,
                   num_devices=N_CORES)
    vtT = nc.dram_tensor("vtT", [E, NTOK], BF16, kind="ExternalInput")
    idxt = nc.dram_tensor("idxt", [3, NTOK], BF16, kind="ExternalInput")
    wt = nc.dram_tensor("wt", [NTOT, E], BF16, kind="ExternalInput")
    thetaT = nc.dram_tensor("thetaT", [E, NTOK], BF16, kind="ExternalOutput")
    ctT = nc.dram_tensor("ctT", [NTOT, NTOK], FP8, kind="ExternalOutput")

    with tile.TileContext(nc) as tc:
        with (
            tc.tile_pool(name="const", bufs=1) as const,
            tc.tile_pool(name="vtp", bufs=8) as vtp,
            tc.tile_pool(name="outp", bufs=3) as outp,
            tc.tile_pool(name="ctp", bufs=3) as ctp,
            tc.tile_pool(name="ct8p", bufs=2) as ct8p,
            tc.tile_pool(name="mmsb", bufs=3) as mmsb,
            tc.tile_pool(name="ps_b", bufs=3, space="PSUM") as ps_b,
            tc.tile_pool(name="ps_m", bufs=5, space="PSUM") as ps_m,
        ):
            # [e, tok] views split the 256 e-rows into 2 x 128 partitions
            vt_view = vtT.ap().rearrange("(h p) t -> p h t", p=128)
            th_view = thetaT.ap().rearrange("(h p) t -> p h t", p=128)
            # index head first on the SP HWDGE ring: it gates the whole
            # compute front-end; the tail streams while pair 0 computes
            idxt_sb = const.tile([3, NTOK], BF16)
            nc.sync.dma_start(idxt_sb[:, 0:PTOK], idxt.ap()[:, 0:PTOK])
            # first vt pair next on the same ring
            vt0 = vtp.tile([128, EH, PTOK], BF16, tag="vt")
            nc.sync.dma_start(vt0[:], vt_view[:, :, 0:PTOK])
            nc.sync.dma_start(idxt_sb[:, PTOK:], idxt.ap()[:, PTOK:])
            # weight on the independent ACT HWDGE ring
            wt_sb = const.tile([NTOT, E], BF16)
            nc.scalar.dma_start(wt_sb[:], wt.ap())
            # device-built constants (no DMA), ahead of the Pool DMA burst
            e3_sb = const.tile([3, NTOT], BF16)
            nc.gpsimd.memset(e3_sb[:], 1.0)
            nc.gpsimd.affine_select(
                out=e3_sb[:].rearrange("p (a b) -> p a b", a=3),
                in_=e3_sb[:].rearrange("p (a b) -> p a b", a=3),
                pattern=[[1, 3], [0, 32]],
                compare_op=mybir.AluOpType.is_equal,
                fill=0.0, base=0, channel_multiplier=-1,
            )
            iota_col = const.tile([NTOT, 1], F32)
            nc.gpsimd.iota(iota_col[:], [[0, 1]], channel_multiplier=1,
                           allow_small_or_imprecise_dtypes=True)
            # all remaining vt pairs up front on SWDGE: loads drain early,
            # leaving the late window to the store stream
            vt_tiles = [vt0]
            for p in range(1, NPAIR):
                t = vtp.tile([128, EH, PTOK], BF16, tag="vt")
                nc.gpsimd.dma_start(
                    t[:], vt_view[:, :, p * PTOK:(p + 1) * PTOK])
                vt_tiles.append(t)

            def bc_mm(g):
                t = ps_b.tile([NTOT, GTOK], F32, tag="bc")
                nc.tensor.matmul(
                    t[:], e3_sb[:], idxt_sb[:, g * GTOK:(g + 1) * GTOK],
                    start=True, stop=True,
                )
                return t

            def compare(g, bc_t, ct_t):
                # transposed one-hot in bf16: an fp8 PE stream keeps the
                # HAM cold (stuck at 1.2GHz); bf16 lets the PE ramp to 2.4
                nc.vector.tensor_scalar(
                    ct_t[:, (g % 2) * GTOK:(g % 2 + 1) * GTOK],
                    bc_t[:], iota_col[:, 0:1], None,
                    mybir.AluOpType.is_equal,
                )

            # software-pipeline prologue: one pair of groups ahead
            bc_tiles = {0: bc_mm(0), 1: bc_mm(1)}
            ct_tiles = {0: ctp.tile([NTOT, PTOK], BF16, tag="ct_t",
                                    name="ct_t_0")}
            compare(0, bc_tiles.pop(0), ct_tiles[0])
            compare(1, bc_tiles.pop(1), ct_tiles[0])

            for p in range(NPAIR):
                ct_t = ct_tiles.pop(p)
                # Cct.T: 4 single-bank matmuls, same-stationary adjacent
                mm_ps = [[None, None], [None, None]]
                for h in range(EH):
                    for g in range(2):
                        t = ps_m.tile([128, GTOK], F32, tag="mm",
                                      name=f"mm_{p}_{h}_{g}")
                        nc.tensor.matmul(
                            t[:], wt_sb[:, h * 128:(h + 1) * 128],
                            ct_t[:, g * GTOK:(g + 1) * GTOK],
                            start=True, stop=True,
                        )
                        mm_ps[g][h] = t
                # keep PE and DVE fed: next pair's broadcast + one-hot
                if p + 1 < NPAIR:
                    a, b = 2 * p + 2, 2 * p + 3
                    bc_tiles[a] = bc_mm(a)
                    bc_tiles[b] = bc_mm(b)
                    nt = ctp.tile([NTOT, PTOK], BF16, tag="ct_t",
                                  name=f"ct_t_{p + 1}")
                    ct_tiles[p + 1] = nt
                    compare(a, bc_tiles.pop(a), nt)
                    compare(b, bc_tiles.pop(b), nt)
                # fp8 copy of the one-hot for the store, on the otherwise
                # idle GpSimd engine; don't queue it behind the theta store
                ct8 = ct8p.tile([NTOT, PTOK], FP8, name=f"ct8_{p}")
                nc.gpsimd.tensor_copy(ct8[:], ct_t[:])
                nc.sync.dma_start(
                    ctT.ap()[:, p * PTOK:(p + 1) * PTOK], ct8[:])

                vt_big = vt_tiles[p]
                th_tile = outp.tile([128, EH, PTOK], BF16)
                for g in range(2):
                    # PSUM -> SBUF bf16 on the Scalar engine so the gate
                    # runs in DVE 2x mode
                    mm_sb = mmsb.tile([128, EH, GTOK], BF16,
                                      name=f"mm_sb_{p}_{g}")
                    for h in range(EH):
                        nc.scalar.copy(mm_sb[:, h, :], mm_ps[g][h][:])
                    nc.vector.tensor_tensor(
                        th_tile[:, :, g * GTOK:(g + 1) * GTOK],
                        vt_big[:, :, g * GTOK:(g + 1) * GTOK],
                        mm_sb[:],
                        mybir.AluOpType.mult,
                    )
                    if p == NPAIR - 1:
                        # endgame: store per group so the final drain is
                        # half-sized and starts right after its gate
                        lo = p * PTOK + g * GTOK
                        nc.sync.dma_start(
                            th_view[:, :, lo:lo + GTOK],
                            th_tile[:, :, g * GTOK:(g + 1) * GTOK])
                if p < NPAIR - 1:
                    nc.sync.dma_start(
                        th_view[:, :, p * PTOK:(p + 1) * PTOK], th_tile[:])

    nc.compile()
    return nc


def _get_nc():
    global _NC
    if _NC is None:
        _NC = _build_nc()
    return _NC


def _host_prep(vt, rgap, sgap, pcount, W):
    import ml_dtypes
    bf16 = ml_dtypes.bfloat16
    vt = np.asarray(vt, dtype=np.float32)
    rgap = np.asarray(rgap)
    sgap = np.asarray(sgap)
    pcount = np.asarray(pcount)
    W = np.asarray(W, dtype=np.float32)
    wt = np.ascontiguousarray(W.T).astype(bf16)     # [96, 256]
    in_maps = []
    for m in range(N_CORES):
        sl = slice(m * BPC, (m + 1) * BPC)
        # token t = p*64 + i maps to column tau = i*128 + p
        idxs = np.stack(
            [rgap[sl].reshape(NTOK),
             sgap[sl].reshape(NTOK) + 32,
             pcount[sl].reshape(NTOK) + 64], axis=0)          # [3, t]
        idxt = np.ascontiguousarray(
            idxs.reshape(3, 128, NCH).transpose(0, 2, 1).reshape(3, NTOK)
        ).astype(bf16)
        vtT = np.ascontiguousarray(
            vt[sl].reshape(128, NCH, E).transpose(2, 1, 0).reshape(E, NTOK)
        ).astype(bf16)
        in_maps.append({"vtT": vtT, "idxt": idxt, "wt": wt})
    return in_maps


def kernel(vt, rgap, sgap, pcount, W, _trace=False, _tmpdir=None):
    nc = _get_nc()
    in_maps = _host_prep(vt, rgap, sgap, pcount, W)
    res = run_bass_kernel_spmd(
        nc, in_maps, list(range(N_CORES)),
        trace=_trace, **({"tmpdir": _tmpdir} if _tmpdir else {}),
    )
    full = np.empty((B, S, E + NTOT), dtype=np.float32)
    for m in range(N_CORES):
        sl = slice(m * BPC, (m + 1) * BPC)
        view = full[sl].reshape(NTOK, E + NTOT)
        thetaT = np.asarray(res.results[m]["thetaT"]).astype(np.float32)
        ct8 = np.asarray(res.results[m]["ctT"]).astype(np.float32)
        view[:, :E] = thetaT.reshape(E, NCH, 128).transpose(2, 1, 0) \
                            .reshape(NTOK, E)
        view[:, E:] = ct8.reshape(NTOT, NCH, 128).transpose(2, 1, 0) \
                         .reshape(NTOK, NTOT)
    if _trace:
        return full, res
    return full


# revision 15
# speedup vs baseline: 1.2981x; 1.2981x over previous
"""Trainium2 Bass kernel for nn_CIntegration_3487513444382 (embedding_lookup).

Computation (per token): ct = concat(onehot(rgap,32), onehot(sgap,32),
onehot(pcount,32)); out = concat(vt * (ct @ W.T), ct).

Strategy: pure data parallel over the batch dim (64 -> 8 per core), with
all device-side tensors transposed to [feature, token] so the rel-err
budget (2e-2) can buy bandwidth: vt is fed as bf16, theta is stored as
bf16, and the one-hot tail is stored as fp8 (0/1 exact). Per core this
moves ~9.3MB instead of ~20MB of f32 traffic.

Per 512-token group: a tiny E3 matmul broadcasts the (offset) indices
to 96 partitions, a DVE compare against an iota column builds the
transposed one-hot in bf16 (an fp8 PE stream keeps the HAM cold), the
PE streams that one-hot through stationary W.T halves (bf16) to
produce Cct.T in PSUM, ScalarE copies it to SBUF as bf16, and a
2x-mode DVE multiply applies the vt gate. GpSimd down-copies the
one-hot to fp8 for the store. All PSUM tiles are single-bank (3 bc
bufs + 5 mm bufs = 8 banks) and the issue order is software-pipelined
one pair ahead so every engine stays fed and the PE holds its warm
2.4GHz p-state. All vt loads are issued up front.
"""
import numpy as np

import concourse.bass as bass
import concourse.tile as tile
from concourse import bacc, mybir
from concourse.bass_utils import run_bass_kernel_spmd

F32 = mybir.dt.float32
BF16 = mybir.dt.bfloat16
FP8 = mybir.dt.float8e4

N_CORES = 8
B, S, E = 64, 1024, 256
BPC = B // N_CORES          # 8 batches per core
NTOK = BPC * S              # 8192 tokens per core
NCH = NTOK // 128           # 64 chunks of 128 tokens
GTOK = 512                  # tokens per compute group
PTOK = 2 * GTOK             # tokens per pair (DMA batch)
NPAIR = NTOK // PTOK        # 8
NTOT = 96                   # one-hot width
EH = E // 128               # 2 e-halves

_NC = None


def _build_nc():
    nc = bacc.Bacc("TRN2", target_bir_lowering=False, debug=False,
                   num_devices=N_CORES)
    vtT = nc.dram_tensor("vtT", [E, NTOK], BF16, kind="ExternalInput")
    idxt = nc.dram_tensor("idxt", [3, NTOK], BF16, kind="ExternalInput")
    wt = nc.dram_tensor("wt", [NTOT, E], BF16, kind="ExternalInput")
    thetaT = nc.dram_tensor("thetaT", [E, NTOK], BF16, kind="ExternalOutput")
    ctT = nc.dram_tensor("ctT", [NTOT, NTOK], FP8, kind="ExternalOutput")

    with tile.TileContext(nc) as tc:
        with (
            tc.tile_pool(name="const", bufs=1) as const,
            tc.tile_pool(name="vtp", bufs=8) as vtp,
            tc.tile_pool(name="outp", bufs=3) as outp,
            tc.tile_pool(name="ctp", bufs=3) as ctp,
            tc.tile_pool(name="mmsb", bufs=3) as mmsb,
            tc.tile_pool(name="ps_b", bufs=3, space="PSUM") as ps_b,
            tc.tile_pool(name="ps_m", bufs=5, space="PSUM") as ps_m,
        ):
            # [e, tok] views split the 256 e-rows into 2 x 128 partitions
            vt_view = vtT.ap().rearrange("(h p) t -> p h t", p=128)
            th_view = thetaT.ap().rearrange("(h p) t -> p h t", p=128)
            # index head first on the SP HWDGE ring: it gates the whole
            # compute front-end; the tail streams while pair 0 computes
            idxt_sb = const.tile([3, NTOK], BF16)
            nc.sync.dma_start(idxt_sb[:, 0:PTOK], idxt.ap()[:, 0:PTOK])
            # first vt pair next on the same ring
            vt0 = vtp.tile([128, EH, PTOK], BF16, tag="vt")
            nc.sync.dma_start(vt0[:], vt_view[:, :, 0:PTOK])
            nc.sync.dma_start(idxt_sb[:, PTOK:], idxt.ap()[:, PTOK:])
            # weight on the independent ACT HWDGE ring
            wt_sb = const.tile([NTOT, E], BF16)
            nc.scalar.dma_start(wt_sb[:], wt.ap())
            # device-built constants (no DMA), ahead of the Pool DMA burst
            e3_sb = const.tile([3, NTOT], BF16)
            nc.gpsimd.memset(e3_sb[:], 1.0)
            nc.gpsimd.affine_select(
                out=e3_sb[:].rearrange("p (a b) -> p a b", a=3),
                in_=e3_sb[:].rearrange("p (a b) -> p a b", a=3),
                pattern=[[1, 3], [0, 32]],
                compare_op=mybir.AluOpType.is_equal,
                fill=0.0, base=0, channel_multiplier=-1,
            )
            iota_col = const.tile([NTOT, 1], F32)
            nc.gpsimd.iota(iota_col[:], [[0, 1]], channel_multiplier=1,
                           allow_small_or_imprecise_dtypes=True)
            # all remaining vt pairs up front on SWDGE: loads drain early,
            # leaving the late window to the store stream
            vt_tiles = [vt0]
            for p in range(1, NPAIR):
                t = vtp.tile([128, EH, PTOK], BF16, tag="vt")
                nc.gpsimd.dma_start(
                    t[:], vt_view[:, :, p * PTOK:(p + 1) * PTOK])
                vt_tiles.append(t)

            def bc_mm(g):
                t = ps_b.tile([NTOT, GTOK], F32, tag="bc")
                nc.tensor.matmul(
                    t[:], e3_sb[:], idxt_sb[:, g * GTOK:(g + 1) * GTOK],
                    start=True, stop=True,
                )
                return t

            def compare(g, bc_t, ct_t):
                # transposed one-hot in bf16: an fp8 PE stream keeps the
                # HAM cold (stuck at 1.2GHz); bf16 lets the PE ramp to 2.4
                nc.vector.tensor_scalar(
                    ct_t[:, (g % 2) * GTOK:(g % 2 + 1) * GTOK],
                    bc_t[:], iota_col[:, 0:1], None,
                    mybir.AluOpType.is_equal,
                )

            # software-pipeline prologue: one pair of groups ahead
            bc_tiles = {0: bc_mm(0), 1: bc_mm(1)}
            ct_tiles = {0: ctp.tile([NTOT, PTOK], BF16, tag="ct_t",
                                    name="ct_t_0")}
            compare(0, bc_tiles.pop(0), ct_tiles[0])
            compare(1, bc_tiles.pop(1), ct_tiles[0])

            for p in range(NPAIR):
                ct_t = ct_tiles.pop(p)
                # Cct.T: 4 single-bank matmuls, same-stationary adjacent
                mm_ps = [[None, None], [None, None]]
                for h in range(EH):
                    for g in range(2):
                        t = ps_m.tile([128, GTOK], F32, tag="mm",
                                      name=f"mm_{p}_{h}_{g}")
                        nc.tensor.matmul(
                            t[:], wt_sb[:, h * 128:(h + 1) * 128],
                            ct_t[:, g * GTOK:(g + 1) * GTOK],
                            start=True, stop=True,
                        )
                        mm_ps[g][h] = t
                # keep PE and DVE fed: next pair's broadcast + one-hot
                if p + 1 < NPAIR:
                    a, b = 2 * p + 2, 2 * p + 3
                    bc_tiles[a] = bc_mm(a)
                    bc_tiles[b] = bc_mm(b)
                    nt = ctp.tile([NTOT, PTOK], BF16, tag="ct_t",
                                  name=f"ct_t_{p + 1}")
                    ct_tiles[p + 1] = nt
                    compare(a, bc_tiles.pop(a), nt)
                    compare(b, bc_tiles.pop(b), nt)
                # ct store with bf16->fp8 cast done by the DMA itself
                # (SWDGE-only feature): zero engine cost, fp8 HBM bytes
                nc.gpsimd.dma_start(
                    ctT.ap()[:, p * PTOK:(p + 1) * PTOK], ct_t[:])

                vt_big = vt_tiles[p]
                th_tile = outp.tile([128, EH, PTOK], BF16)
                for g in range(2):
                    # PSUM -> SBUF bf16 on the Scalar engine so the gate
                    # runs in DVE 2x mode
                    mm_sb = mmsb.tile([128, EH, GTOK], BF16,
                                      name=f"mm_sb_{p}_{g}")
                    for h in range(EH):
                        nc.scalar.copy(mm_sb[:, h, :], mm_ps[g][h][:])
                    nc.vector.tensor_tensor(
                        th_tile[:, :, g * GTOK:(g + 1) * GTOK],
                        vt_big[:, :, g * GTOK:(g + 1) * GTOK],
                        mm_sb[:],
                        mybir.AluOpType.mult,
                    )
                    if p == NPAIR - 1:
                        # endgame: store per group so the final drain is
                        # half-sized and starts right after its gate
                        lo = p * PTOK + g * GTOK
                        nc.sync.dma_start(
                            th_view[:, :, lo:lo + GTOK],
                            th_tile[:, :, g * GTOK:(g + 1) * GTOK])
                if p < NPAIR - 1:
                    nc.sync.dma_start(
                        th_view[:, :, p * PTOK:(p + 1) * PTOK], th_tile[:])

    nc.compile()
    return nc


def _get_nc():
    global _NC
    if _NC is None:
        _NC = _build_nc()
    return _NC


def _host_prep(vt, rgap, sgap, pcount, W):
    import ml_dtypes
    bf16 = ml_dtypes.bfloat16
    vt = np.asarray(vt, dtype=np.float32)
    rgap = np.asarray(rgap)
    sgap = np.asarray(sgap)
    pcount = np.asarray(pcount)
    W = np.asarray(W, dtype=np.float32)
    wt = np.ascontiguousarray(W.T).astype(bf16)     # [96, 256]
    in_maps = []
    for m in range(N_CORES):
        sl = slice(m * BPC, (m + 1) * BPC)
        # token t = p*64 + i maps to column tau = i*128 + p
        idxs = np.stack(
            [rgap[sl].reshape(NTOK),
             sgap[sl].reshape(NTOK) + 32,
             pcount[sl].reshape(NTOK) + 64], axis=0)          # [3, t]
        idxt = np.ascontiguousarray(
            idxs.reshape(3, 128, NCH).transpose(0, 2, 1).reshape(3, NTOK)
        ).astype(bf16)
        vtT = np.ascontiguousarray(
            vt[sl].reshape(128, NCH, E).transpose(2, 1, 0).reshape(E, NTOK)
        ).astype(bf16)
        in_maps.append({"vtT": vtT, "idxt": idxt, "wt": wt})
    return in_maps


def kernel(vt, rgap, sgap, pcount, W, _trace=False, _tmpdir=None):
    nc = _get_nc()
    in_maps = _host_prep(vt, rgap, sgap, pcount, W)
    res = run_bass_kernel_spmd(
        nc, in_maps, list(range(N_CORES)),
        trace=_trace, **({"tmpdir": _tmpdir} if _tmpdir else {}),
    )
    full = np.empty((B, S, E + NTOT), dtype=np.float32)
    for m in range(N_CORES):
        sl = slice(m * BPC, (m + 1) * BPC)
        view = full[sl].reshape(NTOK, E + NTOT)
        thetaT = np.asarray(res.results[m]["thetaT"]).astype(np.float32)
        ct8 = np.asarray(res.results[m]["ctT"]).astype(np.float32)
        view[:, :E] = thetaT.reshape(E, NCH, 128).transpose(2, 1, 0) \
                            .reshape(NTOK, E)
        view[:, E:] = ct8.reshape(NTOT, NCH, 128).transpose(2, 1, 0) \
                         .reshape(NTOK, NTOT)
    if _trace:
        return full, res
    return full


# revision 18
# speedup vs baseline: 1.3412x; 1.0332x over previous
"""Trainium2 Bass kernel for nn_CIntegration_3487513444382 (embedding_lookup).

Computation (per token): ct = concat(onehot(rgap,32), onehot(sgap,32),
onehot(pcount,32)); out = concat(vt * (ct @ W.T), ct).

Strategy: pure data parallel over the batch dim (64 -> 8 per core), with
all device-side tensors transposed to [feature, token] so the rel-err
budget (2e-2) can buy bandwidth: vt is fed as bf16, theta is stored as
bf16, and the one-hot tail is stored as fp8 (0/1 exact). Per core this
moves ~9.3MB instead of ~20MB of f32 traffic.

Per 512-token group: a tiny E3 matmul broadcasts the (offset) indices
to 96 partitions, a DVE compare against an iota column builds the
transposed one-hot in bf16 (an fp8 PE stream keeps the HAM cold), the
PE streams that one-hot through stationary W.T halves (bf16) to
produce Cct.T in PSUM, ScalarE copies it to SBUF as bf16, and a
2x-mode DVE multiply applies the vt gate. GpSimd down-copies the
one-hot to fp8 for the store. All PSUM tiles are single-bank (3 bc
bufs + 5 mm bufs = 8 banks) and the issue order is software-pipelined
one pair ahead so every engine stays fed and the PE holds its warm
2.4GHz p-state. All vt loads are issued up front.
"""
import numpy as np

import concourse.bass as bass
import concourse.tile as tile
from concourse import bacc, mybir
from concourse.bass_utils import run_bass_kernel_spmd

F32 = mybir.dt.float32
BF16 = mybir.dt.bfloat16
FP8 = mybir.dt.float8e4

N_CORES = 8
B, S, E = 64, 1024, 256
BPC = B // N_CORES          # 8 batches per core
NTOK = BPC * S              # 8192 tokens per core
NCH = NTOK // 128           # 64 chunks of 128 tokens
GTOK = 512                  # tokens per compute group
PTOK = 2 * GTOK             # tokens per pair (DMA batch)
NPAIR = NTOK // PTOK        # 8
NTOT = 96                   # one-hot width
EH = E // 128               # 2 e-halves

_NC = None


def _build_nc():
    nc = bacc.Bacc("TRN2", target_bir_lowering=False, debug=False,
                   num_devices=N_CORES)
    vtT = nc.dram_tensor("vtT", [E, NTOK], BF16, kind="ExternalInput")
    idxt = nc.dram_tensor("idxt", [3, NTOK], BF16, kind="ExternalInput")
    wt = nc.dram_tensor("wt", [NTOT, E], BF16, kind="ExternalInput")
    thetaT = nc.dram_tensor("thetaT", [E, NTOK], BF16, kind="ExternalOutput")
    ctT = nc.dram_tensor("ctT", [NTOT, NTOK], FP8, kind="ExternalOutput")

    with tile.TileContext(nc) as tc:
        with (
            tc.tile_pool(name="const", bufs=1) as const,
            tc.tile_pool(name="vtp", bufs=8) as vtp,
            tc.tile_pool(name="outp", bufs=3) as outp,
            tc.tile_pool(name="ctp", bufs=3) as ctp,
            tc.tile_pool(name="mmsb", bufs=3) as mmsb,
            tc.tile_pool(name="ps_b", bufs=3, space="PSUM") as ps_b,
            tc.tile_pool(name="ps_m", bufs=5, space="PSUM") as ps_m,
        ):
            # [e, tok] views split the 256 e-rows into 2 x 128 partitions
            vt_view = vtT.ap().rearrange("(h p) t -> p h t", p=128)
            th_view = thetaT.ap().rearrange("(h p) t -> p h t", p=128)
            # index head first on the SP HWDGE ring: it gates the whole
            # compute front-end; the tail streams while pair 0 computes.
            # idxt is held on 96 partitions (rows 3+ zeroed) so the bc
            # matmul streams 96 active rows: a K=3 stream reads as "idle"
            # to the PE activity monitor and blocks the 2.4GHz up-shift.
            # real index rows live on partitions 93..95; 0..92 are zeros
            # (partition-offset engine access must be quadrant-aligned, so
            # the zero block starts at partition 0)
            idxt_sb = const.tile([NTOT, NTOK], BF16)
            nc.vector.memset(idxt_sb[0:NTOT - 3, :], 0.0)
            nc.sync.dma_start(idxt_sb[NTOT - 3:, 0:PTOK],
                              idxt.ap()[:, 0:PTOK])
            # first vt pair next on the same ring
            vt0 = vtp.tile([128, EH, PTOK], BF16, tag="vt")
            nc.sync.dma_start(vt0[:], vt_view[:, :, 0:PTOK])
            nc.sync.dma_start(idxt_sb[NTOT - 3:, PTOK:], idxt.ap()[:, PTOK:])
            # weight on the independent ACT HWDGE ring
            wt_sb = const.tile([NTOT, E], BF16)
            nc.scalar.dma_start(wt_sb[:], wt.ap())
            # device-built constants (no DMA), ahead of the Pool DMA burst
            e3_sb = const.tile([NTOT, NTOT], BF16)
            nc.gpsimd.memset(e3_sb[:], 1.0)
            nc.gpsimd.affine_select(
                out=e3_sb[:].rearrange("p (a b) -> p a b", a=3),
                in_=e3_sb[:].rearrange("p (a b) -> p a b", a=3),
                pattern=[[1, 3], [0, 32]],
                compare_op=mybir.AluOpType.is_equal,
                fill=0.0, base=NTOT - 3, channel_multiplier=-1,
            )
            iota_col = const.tile([NTOT, 1], F32)
            nc.gpsimd.iota(iota_col[:], [[0, 1]], channel_multiplier=1,
                           allow_small_or_imprecise_dtypes=True)
            # all remaining vt pairs up front on SWDGE: loads drain early,
            # leaving the late window to the store stream
            vt_tiles = [vt0]
            for p in range(1, NPAIR):
                t = vtp.tile([128, EH, PTOK], BF16, tag="vt")
                nc.gpsimd.dma_start(
                    t[:], vt_view[:, :, p * PTOK:(p + 1) * PTOK])
                vt_tiles.append(t)

            def bc_mm(g):
                t = ps_b.tile([NTOT, GTOK], F32, tag="bc")
                nc.tensor.matmul(
                    t[:], e3_sb[:], idxt_sb[:, g * GTOK:(g + 1) * GTOK],
                    start=True, stop=True,
                )
                return t

            def compare(g, bc_t, ct_t):
                # transposed one-hot in bf16: an fp8 PE stream keeps the
                # HAM cold (stuck at 1.2GHz); bf16 lets the PE ramp to 2.4
                nc.vector.tensor_scalar(
                    ct_t[:, (g % 2) * GTOK:(g % 2 + 1) * GTOK],
                    bc_t[:], iota_col[:, 0:1], None,
                    mybir.AluOpType.is_equal,
                )

            # software-pipeline prologue: one pair of groups ahead
            bc_tiles = {0: bc_mm(0), 1: bc_mm(1)}
            ct_tiles = {0: ctp.tile([NTOT, PTOK], BF16, tag="ct_t",
                                    name="ct_t_0")}
            compare(0, bc_tiles.pop(0), ct_tiles[0])
            compare(1, bc_tiles.pop(1), ct_tiles[0])

            for p in range(NPAIR):
                ct_t = ct_tiles.pop(p)
                # Cct.T: 4 single-bank matmuls, same-stationary adjacent
                mm_ps = [[None, None], [None, None]]
                for h in range(EH):
                    for g in range(2):
                        t = ps_m.tile([128, GTOK], F32, tag="mm",
                                      name=f"mm_{p}_{h}_{g}")
                        nc.tensor.matmul(
                            t[:], wt_sb[:, h * 128:(h + 1) * 128],
                            ct_t[:, g * GTOK:(g + 1) * GTOK],
                            start=True, stop=True,
                        )
                        mm_ps[g][h] = t
                # keep PE and DVE fed: next pair's broadcast + one-hot
                if p + 1 < NPAIR:
                    a, b = 2 * p + 2, 2 * p + 3
                    bc_tiles[a] = bc_mm(a)
                    bc_tiles[b] = bc_mm(b)
                    nt = ctp.tile([NTOT, PTOK], BF16, tag="ct_t",
                                  name=f"ct_t_{p + 1}")
                    ct_tiles[p + 1] = nt
                    compare(a, bc_tiles.pop(a), nt)
                    compare(b, bc_tiles.pop(b), nt)
                # ct store with bf16->fp8 cast done by the DMA itself
                # (SWDGE-only feature): zero engine cost, fp8 HBM bytes
                nc.gpsimd.dma_start(
                    ctT.ap()[:, p * PTOK:(p + 1) * PTOK], ct_t[:])

                vt_big = vt_tiles[p]
                th_tile = outp.tile([128, EH, PTOK], BF16)
                for g in range(2):
                    # PSUM -> SBUF bf16 on the Scalar engine so the gate
                    # runs in DVE 2x mode
                    mm_sb = mmsb.tile([128, EH, GTOK], BF16,
                                      name=f"mm_sb_{p}_{g}")
                    for h in range(EH):
                        nc.scalar.copy(mm_sb[:, h, :], mm_ps[g][h][:])
                    nc.vector.tensor_tensor(
                        th_tile[:, :, g * GTOK:(g + 1) * GTOK],
                        vt_big[:, :, g * GTOK:(g + 1) * GTOK],
                        mm_sb[:],
                        mybir.AluOpType.mult,
                    )
                    if p == NPAIR - 1:
                        # endgame: store per group so the final drain is
                        # half-sized and starts right after its gate
                        lo = p * PTOK + g * GTOK
                        nc.sync.dma_start(
                            th_view[:, :, lo:lo + GTOK],
                            th_tile[:, :, g * GTOK:(g + 1) * GTOK])
                if p < NPAIR - 1:
                    nc.sync.dma_start(
                        th_view[:, :, p * PTOK:(p + 1) * PTOK], th_tile[:])

    nc.compile()
    return nc


def _get_nc():
    global _NC
    if _NC is None:
        _NC = _build_nc()
    return _NC


def _host_prep(vt, rgap, sgap, pcount, W):
    import ml_dtypes
    bf16 = ml_dtypes.bfloat16
    vt = np.asarray(vt, dtype=np.float32)
    rgap = np.asarray(rgap)
    sgap = np.asarray(sgap)
    pcount = np.asarray(pcount)
    W = np.asarray(W, dtype=np.float32)
    wt = np.ascontiguousarray(W.T).astype(bf16)     # [96, 256]
    in_maps = []
    for m in range(N_CORES):
        sl = slice(m * BPC, (m + 1) * BPC)
        # token t = p*64 + i maps to column tau = i*128 + p
        idxs = np.stack(
            [rgap[sl].reshape(NTOK),
             sgap[sl].reshape(NTOK) + 32,
             pcount[sl].reshape(NTOK) + 64], axis=0)          # [3, t]
        idxt = np.ascontiguousarray(
            idxs.reshape(3, 128, NCH).transpose(0, 2, 1).reshape(3, NTOK)
        ).astype(bf16)
        vtT = np.ascontiguousarray(
            vt[sl].reshape(128, NCH, E).transpose(2, 1, 0).reshape(E, NTOK)
        ).astype(bf16)
        in_maps.append({"vtT": vtT, "idxt": idxt, "wt": wt})
    return in_maps


def kernel(vt, rgap, sgap, pcount, W, _trace=False, _tmpdir=None):
    nc = _get_nc()
    in_maps = _host_prep(vt, rgap, sgap, pcount, W)
    res = run_bass_kernel_spmd(
        nc, in_maps, list(range(N_CORES)),
        trace=_trace, **({"tmpdir": _tmpdir} if _tmpdir else {}),
    )
    full = np.empty((B, S, E + NTOT), dtype=np.float32)
    for m in range(N_CORES):
        sl = slice(m * BPC, (m + 1) * BPC)
        view = full[sl].reshape(NTOK, E + NTOT)
        thetaT = np.asarray(res.results[m]["thetaT"]).astype(np.float32)
        ct8 = np.asarray(res.results[m]["ctT"]).astype(np.float32)
        view[:, :E] = thetaT.reshape(E, NCH, 128).transpose(2, 1, 0) \
                            .reshape(NTOK, E)
        view[:, E:] = ct8.reshape(NTOT, NCH, 128).transpose(2, 1, 0) \
                         .reshape(NTOK, NTOT)
    if _trace:
        return full, res
    return full
